# revision 1
# baseline (speedup 1.0000x reference)
"""Trainium2 Bass kernel for nn_Attention_5514738008849.

Dense transformer attention block with axial rotary embeddings:
  x:(8,1024,1024) -> qkv -> rope(q,k) -> softmax(qk^T/sqrt(d)) v -> proj+bias

Sharding: pure data-parallel over batch B=8 across the 8 NeuronCores (one
batch element per core, full weights replicated). No collectives.

Per-core dataflow (all matmuls fp32r: fp32 bits, 11-bit mantissa,
1 cycle/row at N>=256 vs 4 for plain fp32):
  - x^T supplied by the host (numpy transpose in kernel()), streamed on the
    SWDGE queue in parallel with weights on HWDGE
  - Q^T,K^T = W^T x^T  (out-dim on partitions); V = x W_v (token-major)
  - rotary: pair-shuffle via a 128x128 signed-permutation matmul, then
    q_rot = q*cos + shuf*sin elementwise on DVE (pass-dims use cos=1,sin=0);
    PSUM evacuations ride on the otherwise-idle Scalar engine
  - logits^T[k,q] per head; exp on ACT (scale=1/8 folded in), software-
    pipelined across head boundaries
  - AV with a ones-column appended to V => row 64 of the psum accumulator
    holds the softmax denominator per q; normalize via DVE reciprocal +
    gpsimd partition_broadcast + tensor_tensor multiply
  - proj token-major with bias added via a K=1 ones-row matmul
"""

import os
import sys

sys.path.insert(0, "/opt/trn_rl_repo")

# This kernel needs the axon-tunneled NeuronCores. A JAX_PLATFORMS=cpu pin
# (used by some harnesses for the jax reference) would prevent the axon
# backend from registering; clearing it here is a no-op when jax has already
# initialized and restores device visibility when it hasn't.
if os.environ.get("JAX_PLATFORMS", "") not in ("", None):
    if "axon" not in os.environ["JAX_PLATFORMS"]:
        os.environ.pop("JAX_PLATFORMS", None)

import numpy as np

import concourse.bass as bass
import concourse.bacc as bacc_mod
import concourse.mybir as mybir
from concourse.bass_utils import run_bass_kernel_spmd
from concourse.tile import TileContext

B, N, C = 8, 1024, 1024
H, D = 16, 64          # heads, head dim
ROT = 32               # rotary dims per head (head_dim // 2)
FH = FW = 32           # token grid for axial rope
NCORES = 8
F32 = mybir.dt.float32
F32R = mybir.dt.float32r


def _host_tables():
    """Rotary cos/sin in d-major (dim-on-partition) layout + shuffle matrix."""
    dim_r = D // 4                                    # 16
    base = np.linspace(1.0, (FH * FW) / 2.0, dim_r // 2) * np.pi   # (8,)

    def axis_freqs(n):
        pos = np.linspace(-1.0, 1.0, n)
        f = pos[:, None] * base[None, :]              # (n, 8)
        return np.repeat(f, 2, axis=-1)               # (n, 16)

    fH = np.broadcast_to(axis_freqs(FH)[:, None, :], (FH, FW, dim_r))
    fW = np.broadcast_to(axis_freqs(FW)[None, :, :], (FH, FW, dim_r))
    freqs = np.concatenate([fH, fW], axis=-1).reshape(N, ROT)      # (1024, 32)

    # d-major table for one 128-partition block = two heads:
    # rows 0-31 rot (head even), 32-63 pass, 64-95 rot (head odd), 96-127 pass
    cos_d = np.ones((128, N), np.float32)
    sin_d = np.zeros((128, N), np.float32)
    ct = np.cos(freqs).T.astype(np.float32)           # (32, 1024)
    st = np.sin(freqs).T.astype(np.float32)
    cos_d[0:32] = ct
    cos_d[64:96] = ct
    sin_d[0:32] = st
    sin_d[64:96] = st

    # signed permutation: shuf[2i] = -q[2i+1], shuf[2i+1] = q[2i] on rot rows
    pshuf = np.zeros((128, 128), np.float32)
    for off in (0, 64):
        for i in range(ROT // 2):
            r0, r1 = off + 2 * i, off + 2 * i + 1
            pshuf[r1, r0] = -1.0                      # out[r0] = -in[r1]
            pshuf[r0, r1] = 1.0                       # out[r1] = +in[r0]

    return cos_d, sin_d, pshuf


def _build_program():
    nc = bacc_mod.Bacc()
    xt_h = nc.declare_dram_parameter("xt", [C, N], F32, isOutput=False)
    wqkv_h = nc.declare_dram_parameter("w_qkv", [C, 3 * C], F32, isOutput=False)
    wproj_h = nc.declare_dram_parameter("w_proj", [C, C], F32, isOutput=False)
    brow_h = nc.declare_dram_parameter("b_row", [1, C], F32, isOutput=False)
    cos_h = nc.declare_dram_parameter("cos_d", [128, N], F32, isOutput=False)
    sin_h = nc.declare_dram_parameter("sin_d", [128, N], F32, isOutput=False)
    pshuf_h = nc.declare_dram_parameter("pshuf", [128, 128], F32, isOutput=False)
    ones_h = nc.declare_dram_parameter("ones_row", [1, 128], F32, isOutput=False)
    onescol_h = nc.declare_dram_parameter("ones_col", [128, 16], F32, isOutput=False)
    out_h = nc.declare_dram_parameter("out", [N, C], F32, isOutput=True)

    def f32r(ap):
        return ap.bitcast(F32R)

    with nc.allow_low_precision(reason="fp32r (11-bit mantissa) operands"), \
         TileContext(nc) as tc, \
         tc.tile_pool(name="consts", bufs=1) as consts, \
         tc.tile_pool(name="big", bufs=1) as big:
        cos_sb = consts.tile([128, N], F32)
        sin_sb = consts.tile([128, N], F32)
        pshuf_sb = consts.tile([128, 128], F32)
        brow_sb = consts.tile([1, C], F32)
        ones_sb = consts.tile([1, 128], F32)
        nc.sync.dma_start(out=cos_sb, in_=cos_h[:, :])
        nc.sync.dma_start(out=sin_sb, in_=sin_h[:, :])
        nc.sync.dma_start(out=f32r(pshuf_sb), in_=f32r(pshuf_h[:, :]))
        nc.sync.dma_start(out=f32r(brow_sb), in_=f32r(brow_h[:, :]))
        nc.sync.dma_start(out=f32r(ones_sb), in_=f32r(ones_h[:, :]))

        # persistent through phases 2-3 (80.25 KB/partition)
        qrot_sb = big.tile([128, 8, N], F32)      # Q_rot^T  (d-major)
        krot_sb = big.tile([128, 8, N], F32)      # K_rot^T
        vext_sb = big.tile([128, 8, 16, 65], F32)  # V | ones, per tok-block

        # ============ phases 1-2 (xT scoped here) ============
        with tc.tile_pool(name="xtp", bufs=1) as xtp:
            xT_sb = xtp.tile([128, 8, N], F32)

            # ---- phase 1: load x^T (host-transposed) on the SWDGE
            # queue so it streams in parallel with w_qkv on HWDGE ----
            for cb in range(8):
                nc.gpsimd.dma_start(
                    out=f32r(xT_sb[:, cb, :]),
                    in_=f32r(xt_h[cb * 128:(cb + 1) * 128, :]),
                )

            # ---- phase 2: QKV + rotary + V_ext ----
            with (
                tc.tile_pool(name="wq", bufs=16) as wq,
                tc.tile_pool(name="rot", bufs=3) as rot,
                tc.tile_pool(name="ps_qkv", bufs=3, space="PSUM") as ps_qkv,
                tc.tile_pool(name="ps_misc", bufs=1, space="PSUM") as ps_misc,
            ):
                for og in (4, 5, 0, 2, 1, 3):     # V first, then Q/K interleaved
                    w_tiles = []
                    for kb in range(8):
                        w_t = wq.tile([128, 512], F32, tag="w_t",
                                      name=f"w_t{og}_{kb}")
                        nc.sync.dma_start(
                            out=f32r(w_t),
                            in_=f32r(wqkv_h[kb * 128:(kb + 1) * 128,
                                            og * 512:(og + 1) * 512]),
                        )
                        w_tiles.append(w_t)

                    if og < 4:                    # Q^T / K^T (d-major)
                        for j in range(4):
                            ob = og * 4 + j       # global 128-out block
                            qkv_ps = ps_qkv.tile([128, N], F32, tag="qkv_ps", name=f"qkv_ps{ob}")
                            for kb in range(8):
                                lhs = w_tiles[kb][:, j * 128:(j + 1) * 128]
                                for qc in range(2):
                                    nc.tensor.matmul(
                                        qkv_ps[:, qc * 512:(qc + 1) * 512],
                                        f32r(lhs),
                                        f32r(xT_sb[:, kb,
                                                   qc * 512:(qc + 1) * 512]),
                                        start=(kb == 0),
                                        stop=(kb == 7),
                                    )
                            dst = (qrot_sb if ob < 8 else krot_sb)
                            hp = ob % 8
                            q_sb = rot.tile([128, N], F32, tag="q_sb")
                            nc.scalar.copy(f32r(q_sb), qkv_ps)
                            shuf_ps = ps_misc.tile([128, N], F32, tag="shuf_ps",
                                                   name=f"shuf{ob}")
                            for qc in range(2):
                                nc.tensor.matmul(
                                    shuf_ps[:, qc * 512:(qc + 1) * 512],
                                    f32r(pshuf_sb),
                                    f32r(q_sb[:, qc * 512:(qc + 1) * 512]),
                                    start=True,
                                    stop=True,
                                )
                            tmp = rot.tile([128, N], F32, tag="tmp")
                            nc.vector.tensor_mul(tmp, shuf_ps, sin_sb)
                            nc.vector.tensor_mul(f32r(dst[:, hp, :]), q_sb, cos_sb)
                            nc.vector.tensor_add(
                                f32r(dst[:, hp, :]), dst[:, hp, :], tmp
                            )
                    else:                         # V half (token-major)
                        vh = og - 4               # 0: heads 0-7, 1: 8-15
                        for tb in range(8):
                            v_ps = ps_qkv.tile([128, 512], F32, tag="qkv_ps", name=f"v_ps{og}_{tb}")
                            for kb in range(8):
                                nc.tensor.matmul(
                                    v_ps,
                                    f32r(xT_sb[:, kb,
                                               tb * 128:(tb + 1) * 128]),
                                    f32r(w_tiles[kb]),
                                    start=(kb == 0),
                                    stop=(kb == 7),
                                )
                            nc.scalar.copy(
                                f32r(vext_sb[:, tb, vh * 8:(vh + 1) * 8, 0:64]),
                                v_ps.rearrange("p (a b) -> p a b", a=8),
                            )
                        if vh == 1:
                            for tb in range(8):
                                nc.sync.dma_start(
                                    out=f32r(vext_sb[:, tb, :, 64:65]),
                                    in_=f32r(onescol_h[:, :]),
                                )

        # ============ phases 3-4 (attn scoped here) ============
        with tc.tile_pool(name="attnp", bufs=1) as attnp:
            attn_sb = attnp.tile([128, 8, N], F32)   # attn_out^T (c-major)

            # ---- phase 3: attention, head pairs (adjacent K=64 matmuls
            # at base partitions 0/64 row-pack on the PE) ----
            with tc.tile_pool(name="wpre", bufs=8) as wpre:
                # prefetch w_proj rows during attention
                wp_tiles = []
                for cb in range(8):
                    wp_t = wpre.tile([128, C], F32, tag="wp_t", name=f"wp{cb}")
                    nc.sync.dma_start(
                        out=f32r(wp_t),
                        in_=f32r(wproj_h[cb * 128:(cb + 1) * 128, :]),
                    )
                    wp_tiles.append(wp_t)

                with (
                    tc.tile_pool(name="expp", bufs=4) as expp,
                    tc.tile_pool(name="navp", bufs=2) as navp,
                    tc.tile_pool(name="ps_lg", bufs=2, space="PSUM") as ps_lg,
                    tc.tile_pool(name="ps_av", bufs=2, space="PSUM") as ps_av,
                ):
                    def emit_logits(h, kt):
                        hp, r0 = h // 2, (h % 2) * 64
                        lg_ps = ps_lg.tile([128, N], F32, tag="lg_ps",
                                           name=f"lg{h}_{kt}")
                        lhs = krot_sb[r0:r0 + 64, hp,
                                      kt * 128:(kt + 1) * 128]
                        for qc in range(2):
                            nc.tensor.matmul(
                                lg_ps[:, qc * 512:(qc + 1) * 512],
                                f32r(lhs),
                                f32r(qrot_sb[r0:r0 + 64, hp,
                                             qc * 512:(qc + 1) * 512]),
                                start=True,
                                stop=True,
                            )
                        return lg_ps

                    lg_next = None
                    for h in range(H):
                        hp, r0 = h // 2, (h % 2) * 64
                        av_ps = ps_av.tile([65, N], F32, tag="av_ps",
                                           name=f"av{h}")
                        for kt in range(8):
                            if lg_next is not None:
                                lg_ps, lg_next = lg_next, None
                            else:
                                lg_ps = emit_logits(h, kt)
                            if kt == 7 and h + 1 < H:
                                # pre-issue next head's first logits so the
                                # ACT pipe never drains at head boundaries
                                lg_next = emit_logits(h + 1, 0)
                            e_sb = expp.tile([128, N], F32, tag="e_sb",
                                             name=f"e{h}_{kt}")
                            nc.scalar.activation(
                                f32r(e_sb), lg_ps,
                                mybir.ActivationFunctionType.Exp, scale=0.125,
                            )
                            for qc in range(2):
                                nc.tensor.matmul(
                                    av_ps[:, qc * 512:(qc + 1) * 512],
                                    f32r(vext_sb[:, kt, h, :]),
                                    f32r(e_sb[:, qc * 512:(qc + 1) * 512]),
                                    start=(kt == 0),
                                    stop=(kt == 7),
                                )
                        recip = navp.tile([1, N], F32, tag="recip", bufs=1)
                        nc.vector.reciprocal(recip, av_ps[64:65, :])
                        av_sb = navp.tile([64, N], F32, tag="av_sb")
                        nc.vector.tensor_copy(av_sb, av_ps[0:64, :])
                        # broadcast 1/rowsum across partitions on gpsimd
                        rb_sb = navp.tile([64, N], F32, tag="rb_sb", bufs=1)
                        nc.gpsimd.partition_broadcast(rb_sb, recip)
                        nc.vector.tensor_mul(
                            f32r(attn_sb[r0:r0 + 64, hp, :]), av_sb, rb_sb
                        )

                # ---- phase 4: proj + bias (weights prefetched) ----
                with (
                    tc.tile_pool(name="yout", bufs=2) as yout,
                    tc.tile_pool(name="ps_y", bufs=4, space="PSUM") as ps_y,
                ):
                    for qg in range(2):               # 4 q-blocks per group
                        y_tiles = [
                            ps_y.tile([128, C], F32, tag="y_ps",
                                      name=f"y_ps{qg}_{i}")
                            for i in range(4)
                        ]
                        for cb in range(8):
                            for i in range(4):
                                qb = qg * 4 + i
                                lhs = attn_sb[:, cb, qb * 128:(qb + 1) * 128]
                                for oc in range(2):
                                    nc.tensor.matmul(
                                        y_tiles[i][:, oc * 512:(oc + 1) * 512],
                                        f32r(lhs),
                                        f32r(wp_tiles[cb][:,
                                             oc * 512:(oc + 1) * 512]),
                                        start=(cb == 0),
                                        stop=False,
                                    )
                        for i in range(4):
                            qb = qg * 4 + i
                            for oc in range(2):
                                nc.tensor.matmul(
                                    y_tiles[i][:, oc * 512:(oc + 1) * 512],
                                    f32r(ones_sb),
                                    f32r(brow_sb[:, oc * 512:(oc + 1) * 512]),
                                    start=False,
                                    stop=True,
                                )
                            y_sb = yout.tile([128, C], F32, tag="y_sb")
                            nc.scalar.copy(y_sb, y_tiles[i])
                            nc.sync.dma_start(
                                out=out_h[qb * 128:(qb + 1) * 128, :], in_=y_sb
                            )
    nc.finalize()
    return nc


_PROGRAM = None


def kernel(x, w_qkv, w_proj, b_proj):
    global _PROGRAM
    if _PROGRAM is None:
        _PROGRAM = _build_program()
    nc = _PROGRAM

    cos_d, sin_d, pshuf = _host_tables()
    shared = {
        "w_qkv": np.ascontiguousarray(w_qkv, np.float32),
        "w_proj": np.ascontiguousarray(w_proj, np.float32),
        "b_row": np.ascontiguousarray(b_proj, np.float32).reshape(1, C),
        "cos_d": cos_d,
        "sin_d": sin_d,
        "pshuf": pshuf,
        "ones_row": np.ones((1, 128), np.float32),
        "ones_col": np.ones((128, 16), np.float32),
    }
    in_maps = [
        {"xt": np.ascontiguousarray(np.asarray(x[b], np.float32).T), **shared}
        for b in range(NCORES)
    ]
    res = run_bass_kernel_spmd(nc, in_maps, core_ids=list(range(NCORES)))
    return np.stack([res.results[b]["out"] for b in range(NCORES)], axis=0)


if __name__ == "__main__":
    xs = np.random.randn(B, N, C).astype(np.float32)
    wq = (np.random.randn(C, 3 * C) / np.sqrt(C)).astype(np.float32)
    wp = (np.random.randn(C, C) / np.sqrt(C)).astype(np.float32)
    bp = (np.random.randn(C) * 0.01).astype(np.float32)
    out = kernel(x=xs, w_qkv=wq, w_proj=wp, b_proj=bp)
    print(out.shape, out.dtype)



# revision 11
# speedup vs baseline: 1.3174x; 1.3174x over previous
"""Trainium2 Bass kernel for nn_Attention_5514738008849.

Dense transformer attention block with axial rotary embeddings:
  x:(8,1024,1024) -> qkv -> rope(q,k) -> softmax(qk^T/sqrt(d)) v -> proj+bias

Sharding: pure data-parallel over batch B=8 across the 8 NeuronCores (one
batch element per core, full weights replicated). No collectives.

Per-core dataflow:
  - QKV runs as fp8e4 DoubleRow matmuls (0.5 cyc/row, K=256 per pass) using a
    3-term hi/lo residual split of both x and w_qkv (host-precomputed):
        x@w ~= x_hi@w_hi + (x_lo@w_hi)/64 + (x_hi/64)@(w_lo*64)
    with w globally prescaled by 16 for fp8 range; terms 1+3 accumulate in one
    PSUM, term 2 in a second, merged at evacuation on GpSimd with the 1/64.
  - rotary: DVE stream_shuffle pair-swaps partitions; the sign and the 1/16
    w-descale fold into the host cos/sin tables; bf16 combine runs at DVE 2x.
  - logits^T[k,q] per head in bf16; exp on ACT (scale=1/8) -- ACT runs only
    the exps (its throughput is the attention-phase floor), all PSUM
    evacuations ride on GpSimd.
  - AV in bf16 with a 16.0-column appended to V so row 64 of the accumulator
    carries the (16x-scaled) softmax denominator; normalize = DVE reciprocal +
    GpSimd partition_broadcast + DVE multiply straight out of PSUM.
  - attention runs in q-halves: during half 1, proj of half 0 fills the PE
    while ACT exps; QKV for heads 8-15 interleaves into heads 0-7's half 0.
  - proj token-major in bf16; bias fused into the GpSimd PSUM evacuation.
"""

import os
import sys

sys.path.insert(0, "/opt/trn_rl_repo")

# This kernel needs the axon-tunneled NeuronCores. A JAX_PLATFORMS=cpu pin
# (used by some harnesses for the jax reference) would prevent the axon
# backend from registering; clearing it here is a no-op when jax has already
# initialized and restores device visibility when it hasn't.
if os.environ.get("JAX_PLATFORMS", "") not in ("", None):
    if "axon" not in os.environ["JAX_PLATFORMS"]:
        os.environ.pop("JAX_PLATFORMS", None)

import numpy as np
import ml_dtypes

import concourse.bass as bass
import concourse.bacc as bacc_mod
import concourse.mybir as mybir
from concourse.bass_utils import run_bass_kernel_spmd
from concourse.tile import TileContext

B, N, C = 8, 1024, 1024
H, D = 16, 64          # heads, head dim
ROT = 32               # rotary dims per head (head_dim // 2)
FH = FW = 32           # token grid for axial rope
NCORES = 8
F32 = mybir.dt.float32
F32R = mybir.dt.float32r
BF16 = mybir.dt.bfloat16
FP8 = mybir.dt.float8e4
U8 = mybir.dt.uint8
U16 = mybir.dt.uint16

SW = 16.0              # global w_qkv prescale for fp8 range
SL = 64.0              # hi/lo residual scale

PAIRMASK = [i ^ 1 for i in range(32)]   # stream_shuffle partition pair swap


def _host_tables():
    """Rotary cos/sin tables, d-major (dim-on-partition), bf16.

    The stream_shuffle is a plain pair swap, so the rotate-half sign lives in
    the sin table (-sin on even rows, +sin on odd rows), and the 1/SW descale
    of the fp8-scaled QKV results is folded into both tables.
    """
    dim_r = D // 4                                    # 16
    base = np.linspace(1.0, (FH * FW) / 2.0, dim_r // 2) * np.pi   # (8,)

    def axis_freqs(n):
        pos = np.linspace(-1.0, 1.0, n)
        f = pos[:, None] * base[None, :]              # (n, 8)
        return np.repeat(f, 2, axis=-1)               # (n, 16)

    fH = np.broadcast_to(axis_freqs(FH)[:, None, :], (FH, FW, dim_r))
    fW = np.broadcast_to(axis_freqs(FW)[None, :, :], (FH, FW, dim_r))
    freqs = np.concatenate([fH, fW], axis=-1).reshape(N, ROT)      # (1024, 32)

    cos_d = np.full((128, N), 1.0 / SW, np.float32)
    sin_d = np.zeros((128, N), np.float32)
    ct = np.cos(freqs).T.astype(np.float32) / SW      # (32, 1024)
    st = np.sin(freqs).T.astype(np.float32) / SW
    sgn = np.where(np.arange(ROT) % 2 == 0, -1.0, 1.0)[:, None].astype(np.float32)
    cos_d[0:32] = ct
    cos_d[64:96] = ct
    sin_d[0:32] = st * sgn
    sin_d[64:96] = st * sgn
    return (cos_d.astype(ml_dtypes.bfloat16).view(np.uint16),
            sin_d.astype(ml_dtypes.bfloat16).view(np.uint16))


def _pair_layout(a):
    """[1024, X] c-major -> [128, 4, 2, X] (partition, k-block-pair, tile)."""
    X = a.shape[1]
    return np.ascontiguousarray(a.reshape(4, 2, 128, X).transpose(2, 0, 1, 3))


def _fp8_split_x(a):
    """hi, unscaled residual, and a/SL, all e4m3.

    The residual stays unscaled so all three QKV terms accumulate into one
    PSUM at the same scale (the w residual is SL-scaled against a/SL).
    """
    f8 = ml_dtypes.float8_e4m3fn
    hi = a.astype(f8)
    lo = (a - hi.astype(np.float32)).astype(f8)
    sm = (a / SL).astype(f8)
    return hi, lo, sm


def _fp8_split_w(a):
    """hi and SL-scaled residual, e4m3 (pairs with the a/SL x operand)."""
    f8 = ml_dtypes.float8_e4m3fn
    hi = a.astype(f8)
    lo = ((a - hi.astype(np.float32)) * SL).astype(f8)
    return hi, lo


def _build_program():
    nc = bacc_mod.Bacc()
    xh_h = nc.declare_dram_parameter("x_hi", [128, 8192], U8, isOutput=False)
    xl_h = nc.declare_dram_parameter("x_lo", [128, 8192], U8, isOutput=False)
    xs_h = nc.declare_dram_parameter("x_sm", [128, 8192], U8, isOutput=False)
    wh_h = nc.declare_dram_parameter("w_hi", [6, 128, 4096], U8, isOutput=False)
    wl_h = nc.declare_dram_parameter("w_lo", [6, 128, 4096], U8, isOutput=False)
    wp_h = nc.declare_dram_parameter("w_proj16", [C, C], U16, isOutput=False)
    brow_h = nc.declare_dram_parameter("b_row", [1, C], F32, isOutput=False)
    cos_h = nc.declare_dram_parameter("cos_d", [128, N], U16, isOutput=False)
    sin_h = nc.declare_dram_parameter("sin_d", [128, N], U16, isOutput=False)
    out_h = nc.declare_dram_parameter("out", [N, C], F32, isOutput=True)

    def f32r(ap):
        return ap.bitcast(F32R)

    DR = mybir.MatmulPerfMode.DoubleRow
    MUL = mybir.AluOpType.mult
    ADD = mybir.AluOpType.add

    with nc.allow_low_precision(reason="fp8/bf16 operands within rel-err gate"), \
         TileContext(nc) as tc, \
         tc.tile_pool(name="consts", bufs=1) as consts, \
         tc.tile_pool(name="big", bufs=1) as big, \
         tc.tile_pool(name="wq", bufs=3) as wq, \
         tc.tile_pool(name="rot", bufs=2) as rot, \
         tc.tile_pool(name="expp", bufs=6) as expp, \
         tc.tile_pool(name="navp", bufs=2) as navp, \
         tc.tile_pool(name="yout", bufs=2) as yout:

        cos_sb = consts.tile([128, N], BF16)
        sin_sb = consts.tile([128, N], BF16)
        brow_sb = consts.tile([1, C], F32)
        bias_bc = consts.tile([128, C], F32)

        # persistent activations
        xh_sb = big.tile([128, 4, 2, N], FP8)
        xl_sb = big.tile([128, 4, 2, N], FP8)
        xs_sb = big.tile([128, 4, 2, N], FP8)
        qrot_sb = big.tile([128, 8, N], BF16)      # Q_rot^T  (d-major)
        krot_sb = big.tile([128, 8, N], BF16)      # K_rot^T
        vext_sb = big.tile([128, 8, 16, 65], BF16)  # V | SW, per tok-block
        attn_sb = big.tile([128, 8, N], BF16)      # attn_out^T (c-major)
        wp_sb = big.tile([128, 8, C], BF16)        # w_proj rows

        # ---- DMA stream (sync/HWDGE, ordered = arrival order) ----
        def dma_x(dst, src):
            for kbp in range(4):
                nc.sync.dma_start(
                    out=dst[:, kbp, :, :].rearrange("p a b -> p (a b)").bitcast(U8),
                    in_=src[:, kbp * 2048:(kbp + 1) * 2048],
                )

        w_tiles = {}

        def dma_w(og):
            whi = wq.tile([128, 4, 2, 512], FP8, tag="whi", name=f"whi{og}")
            wlo = wq.tile([128, 4, 2, 512], FP8, tag="wlo", name=f"wlo{og}")
            nc.sync.dma_start(
                out=whi.rearrange("p a b c -> p (a b c)").bitcast(U8),
                in_=wh_h[og, :, :],
            )
            nc.sync.dma_start(
                out=wlo.rearrange("p a b c -> p (a b c)").bitcast(U8),
                in_=wl_h[og, :, :],
            )
            w_tiles[og] = (whi, wlo)

        dma_x(xh_sb, xh_h)
        dma_w(4)
        dma_x(xl_sb, xl_h)
        dma_x(xs_sb, xs_h)
        dma_w(5)
        nc.sync.dma_start(out=cos_sb.bitcast(U16), in_=cos_h[:, :])
        nc.sync.dma_start(out=sin_sb.bitcast(U16), in_=sin_h[:, :])
        nc.sync.dma_start(out=f32r(brow_sb), in_=f32r(brow_h[:, :]))
        dma_w(0)
        dma_w(2)
        dma_w(1)
        dma_w(3)
        for cb in range(8):
            nc.sync.dma_start(
                out=wp_sb[:, cb, :].bitcast(U16),
                in_=wp_h[cb * 128:(cb + 1) * 128, :],
            )

        # ones(SW) column of V_ext; bias broadcast row
        nc.gpsimd.memset(vext_sb[:, :, :, 64:65], SW)
        nc.gpsimd.partition_broadcast(bias_bc, brow_sb)

        with tc.tile_pool(name="ps_lg", bufs=2, space="PSUM") as ps_lg, \
             tc.tile_pool(name="ps_av", bufs=2, space="PSUM") as ps_av:

            # ---------- QKV (fp8 DoubleRow, 3 terms, one PSUM) ----------
            def qkv_block(ps_m, og, j, col0):
                """One [128, 512] out chunk.

                q/k ogs (0-3): out dims = w cols (j), cols = tokens col0..+512.
                v ogs (4,5): out dims = tokens (j = tb), cols = w cols col0..+512.
                """
                whi, wlo = w_tiles[og]
                qk = og < 4
                for cc in range(2):
                    dm = ps_m[:, cc * 256:(cc + 1) * 256]
                    if qk:
                        wsl = lambda w: w[:, kbp, :, j * 128:(j + 1) * 128]
                        xsl = lambda x: x[:, kbp, :, col0 + cc * 256:col0 + (cc + 1) * 256]
                        terms = [(whi, xh_sb), (whi, xl_sb), (wlo, xs_sb)]
                    else:
                        xsl = lambda x: x[:, kbp, :, j * 128:(j + 1) * 128]
                        wsl = lambda w: w[:, kbp, :, col0 + cc * 256:col0 + (cc + 1) * 256]
                        terms = [(xh_sb, whi), (xl_sb, whi), (xs_sb, wlo)]
                    for ti, (lt, rt) in enumerate(terms):
                        for kbp in range(4):
                            lhs = wsl(lt) if qk else xsl(lt)
                            rhs = xsl(rt) if qk else wsl(rt)
                            nc.tensor.matmul(
                                dm, lhs, rhs,
                                start=(ti == 0 and kbp == 0),
                                stop=(ti == 2 and kbp == 3),
                                perf_mode=DR,
                            )

            def rotary(q_sb, dst):
                """q_sb [128,1024] bf16 (SW-scaled) -> dst = rope(q)/SW."""
                shuf = rot.tile([128, N], BF16, tag="shuf")
                nc.vector.stream_shuffle(shuf, q_sb, PAIRMASK)
                tmp = rot.tile([128, N], BF16, tag="tmp")
                nc.vector.tensor_mul(tmp, shuf, sin_sb)
                nc.vector.tensor_mul(dst, q_sb, cos_sb)
                nc.vector.tensor_add(dst, dst, tmp)

            # ---------- attention ----------
            def attention(h, sig, fillers):
                hp, r0 = h // 2, (h % 2) * 64
                q0 = sig * 512
                es = []
                for ktp in range(4):
                    lg = ps_lg.tile([128, 2, 512], F32, tag="lg",
                                    name=f"lg{h}_{sig}_{ktp}")
                    for i in range(2):
                        kt = ktp * 2 + i
                        nc.tensor.matmul(
                            lg[:, i, :],
                            krot_sb[r0:r0 + 64, hp, kt * 128:(kt + 1) * 128],
                            qrot_sb[r0:r0 + 64, hp, q0:q0 + 512],
                            start=True, stop=True,
                        )
                    e = expp.tile([128, 2, 512], BF16, tag="e",
                                  name=f"e{h}_{sig}_{ktp}")
                    nc.scalar.activation(
                        e.rearrange("p a b -> p (a b)"),
                        lg.rearrange("p a b -> p (a b)"),
                        mybir.ActivationFunctionType.Exp, scale=0.125,
                    )
                    es.append(e)
                    if fillers and ktp % 2 == 1:
                        fillers.pop(0)()
                av = ps_av.tile([65, 512], F32, tag="av", name=f"av{h}_{sig}")
                for ktp in range(4):
                    for i in range(2):
                        kt = ktp * 2 + i
                        nc.tensor.matmul(
                            av, vext_sb[:, kt, h, 0:65], es[ktp][:, i, :],
                            start=(kt == 0), stop=(kt == 7),
                        )
                recip = navp.tile([1, 512], F32, tag="recip", bufs=1)
                nc.vector.reciprocal(recip, av[64:65, :])
                rb = navp.tile([64, 512], F32, tag="rb", bufs=1)
                nc.gpsimd.partition_broadcast(rb, recip)
                nc.vector.tensor_mul(
                    attn_sb[r0:r0 + 64, hp, q0:q0 + 512], av[0:64, :], rb
                )

            # ---------- era A: qkv + attention half 0 ----------
            with tc.tile_pool(name="ps_qm", bufs=2, space="PSUM") as ps_qm:

                def v_unit(og, tb):
                    def emit():
                        m = ps_qm.tile([128, 512], F32, tag="qm", name=f"vm{og}_{tb}")
                        qkv_block(m, og, tb, 0)
                        vh = og - 4
                        nc.scalar.copy(
                            vext_sb[:, tb, vh * 8:(vh + 1) * 8, 0:64],
                            m.rearrange("p (a b) -> p a b", a=8),
                        )
                    return emit

                qsb_tiles = {}

                def qk_unit(og, j, half):
                    """half 0/1 of tokens for q/k out-block j; rotary on half 1."""
                    def emit():
                        m = ps_qm.tile([128, 512], F32, tag="qm", name=f"qm{og}_{j}_{half}")
                        qkv_block(m, og, j, half * 512)
                        if half == 0:
                            qsb_tiles[(og, j)] = rot.tile(
                                [128, N], BF16, tag="q_sb",
                                name=f"qsb{og}_{j}", bufs=2)
                        q_sb = qsb_tiles[(og, j)]
                        nc.scalar.copy(q_sb[:, half * 512:half * 512 + 512], m)
                        if half == 1:
                            dst = (qrot_sb if og in (0, 1) else krot_sb)
                            hp = j + (4 if og in (1, 3) else 0)
                            rotary(q_sb, dst[:, hp, :])
                    return emit

                # V first, then q/k heads 0-7
                for og in (4, 5):
                    for tb in range(8):
                        v_unit(og, tb)()
                for j in range(4):
                    for og in (0, 2):
                        qk_unit(og, j, 0)()
                        qk_unit(og, j, 1)()

                # heads 0-7 half 0, with q/k heads 8-15 as PE fillers
                fillers = []
                for j in range(4):
                    for og in (1, 3):
                        fillers.append(qk_unit(og, j, 0))
                        fillers.append(qk_unit(og, j, 1))
                for h in range(8):
                    attention(h, 0, fillers)
                while fillers:
                    fillers.pop(0)()
                for h in range(8, 16):
                    attention(h, 0, [])

            # ---------- era B: attention half 1 + proj ----------
            with tc.tile_pool(name="ps_y", bufs=1, space="PSUM") as ps_y:

                def proj_chunk(ctx, cb):
                    y, qb = ctx
                    for oc in range(2):
                        nc.tensor.matmul(
                            y[:, oc, :],
                            attn_sb[:, cb, qb * 128:(qb + 1) * 128],
                            wp_sb[:, cb, oc * 512:(oc + 1) * 512],
                            start=(cb == 0), stop=(cb == 7),
                        )
                    if cb == 7:
                        y_sb = yout.tile([128, C], F32, tag="y_sb", name=f"ysb{qb}")
                        nc.vector.scalar_tensor_tensor(
                            out=y_sb, in0=y.rearrange("p a b -> p (a b)"),
                            scalar=1.0, in1=bias_bc, op0=MUL, op1=ADD,
                        )
                        nc.sync.dma_start(
                            out=out_h[qb * 128:(qb + 1) * 128, :], in_=y_sb
                        )

                fillers = []
                for qb in range(4):
                    ctx = None
                    for cb in range(8):
                        def emit(qb=qb, cb=cb):
                            nonlocal ctx
                            if cb == 0:
                                ctx = (ps_y.tile([128, 2, 512], F32, tag="y",
                                                 name=f"y{qb}"), qb)
                            proj_chunk(ctx, cb)
                        fillers.append(emit)
                for h in range(16):
                    attention(h, 1, fillers)
                while fillers:
                    fillers.pop(0)()
                for qb in range(4, 8):
                    y = ps_y.tile([128, 2, 512], F32, tag="y", name=f"y{qb}")
                    for cb in range(8):
                        proj_chunk((y, qb), cb)

    nc.finalize()
    return nc


_PROGRAM = None


def kernel(x, w_qkv, w_proj, b_proj):
    global _PROGRAM
    if _PROGRAM is None:
        _PROGRAM = _build_program()
    nc = _PROGRAM

    cos_d, sin_d = _host_tables()
    wq_s = np.asarray(w_qkv, np.float32) * SW
    whs, wls = [], []
    for og in range(6):
        wro = _pair_layout(wq_s[:, og * 512:(og + 1) * 512])
        hi, lo = _fp8_split_w(wro)
        whs.append(hi.reshape(128, 4096).view(np.uint8))
        wls.append(lo.reshape(128, 4096).view(np.uint8))
    shared = {
        "w_hi": np.ascontiguousarray(np.stack(whs)),
        "w_lo": np.ascontiguousarray(np.stack(wls)),
        "w_proj16": np.ascontiguousarray(
            np.asarray(w_proj, np.float32).astype(ml_dtypes.bfloat16).view(np.uint16)),
        "b_row": np.ascontiguousarray(b_proj, np.float32).reshape(1, C),
        "cos_d": cos_d,
        "sin_d": sin_d,
    }
    in_maps = []
    for b in range(NCORES):
        xr = _pair_layout(np.ascontiguousarray(np.asarray(x[b], np.float32).T))
        hi, lo, sm = _fp8_split_x(xr)
        in_maps.append({
            "x_hi": np.ascontiguousarray(hi.reshape(128, 8192).view(np.uint8)),
            "x_lo": np.ascontiguousarray(lo.reshape(128, 8192).view(np.uint8)),
            "x_sm": np.ascontiguousarray(sm.reshape(128, 8192).view(np.uint8)),
            **shared,
        })
    res = run_bass_kernel_spmd(nc, in_maps, core_ids=list(range(NCORES)))
    return np.stack([res.results[b]["out"] for b in range(NCORES)], axis=0)


if __name__ == "__main__":
    xs = np.random.randn(B, N, C).astype(np.float32)
    wq = (np.random.randn(C, 3 * C) / np.sqrt(C)).astype(np.float32)
    wp = (np.random.randn(C, C) / np.sqrt(C)).astype(np.float32)
    bp = (np.random.randn(C) * 0.01).astype(np.float32)
    out = kernel(x=xs, w_qkv=wq, w_proj=wp, b_proj=bp)
    print(out.shape, out.dtype)


# revision 16
# speedup vs baseline: 1.3609x; 1.0330x over previous
"""Trainium2 Bass kernel for nn_Attention_5514738008849.

Dense transformer attention block with axial rotary embeddings:
  x:(8,1024,1024) -> qkv -> rope(q,k) -> softmax(qk^T/sqrt(d)) v -> proj+bias

Sharding: pure data-parallel over batch B=8 across the 8 NeuronCores (one
batch element per core, full weights replicated). No collectives.

Per-core dataflow:
  - QKV runs as fp8e4 DoubleRow matmuls (0.5 cyc/row, K=256 per pass) using a
    3-term hi/lo residual split of both x and w_qkv (host-precomputed):
        x@w ~= x_hi@w_hi + (x_lo@w_hi)/64 + (x_hi/64)@(w_lo*64)
    with w globally prescaled by 16 for fp8 range; terms 1+3 accumulate in one
    PSUM, term 2 in a second, merged at evacuation on GpSimd with the 1/64.
  - rotary: DVE stream_shuffle pair-swaps partitions; the sign and the 1/16
    w-descale fold into the host cos/sin tables; bf16 combine runs at DVE 2x.
  - logits^T[k,q] per head in bf16; exp on ACT (scale=1/8) -- ACT runs only
    the exps (its throughput is the attention-phase floor), all PSUM
    evacuations ride on GpSimd.
  - AV in bf16 with a 16.0-column appended to V so row 64 of the accumulator
    carries the (16x-scaled) softmax denominator; normalize = DVE reciprocal +
    GpSimd partition_broadcast + DVE multiply straight out of PSUM.
  - attention runs in q-halves: during half 1, proj of half 0 fills the PE
    while ACT exps; QKV for heads 8-15 interleaves into heads 0-7's half 0.
  - proj token-major in bf16; bias fused into the GpSimd PSUM evacuation.
"""

import os
import sys

sys.path.insert(0, "/opt/trn_rl_repo")

# This kernel needs the axon-tunneled NeuronCores. A JAX_PLATFORMS=cpu pin
# (used by some harnesses for the jax reference) would prevent the axon
# backend from registering; clearing it here is a no-op when jax has already
# initialized and restores device visibility when it hasn't.
if os.environ.get("JAX_PLATFORMS", "") not in ("", None):
    if "axon" not in os.environ["JAX_PLATFORMS"]:
        os.environ.pop("JAX_PLATFORMS", None)

import numpy as np
import ml_dtypes

import concourse.bass as bass
import concourse.bacc as bacc_mod
import concourse.mybir as mybir
from concourse.bass_utils import run_bass_kernel_spmd
from concourse.tile import TileContext

B, N, C = 8, 1024, 1024
H, D = 16, 64          # heads, head dim
ROT = 32               # rotary dims per head (head_dim // 2)
FH = FW = 32           # token grid for axial rope
NCORES = 8
F32 = mybir.dt.float32
F32R = mybir.dt.float32r
BF16 = mybir.dt.bfloat16
FP8 = mybir.dt.float8e4
U8 = mybir.dt.uint8
U16 = mybir.dt.uint16

SW = 16.0              # global w_qkv prescale for fp8 range
SL = 64.0              # hi/lo residual scale

PAIRMASK = [i ^ 1 for i in range(32)]   # stream_shuffle partition pair swap


def _host_tables():
    """Rotary cos/sin tables, d-major (dim-on-partition), bf16.

    The stream_shuffle is a plain pair swap, so the rotate-half sign lives in
    the sin table (-sin on even rows, +sin on odd rows), and the 1/SW descale
    of the fp8-scaled QKV results is folded into both tables.
    """
    dim_r = D // 4                                    # 16
    base = np.linspace(1.0, (FH * FW) / 2.0, dim_r // 2) * np.pi   # (8,)

    def axis_freqs(n):
        pos = np.linspace(-1.0, 1.0, n)
        f = pos[:, None] * base[None, :]              # (n, 8)
        return np.repeat(f, 2, axis=-1)               # (n, 16)

    fH = np.broadcast_to(axis_freqs(FH)[:, None, :], (FH, FW, dim_r))
    fW = np.broadcast_to(axis_freqs(FW)[None, :, :], (FH, FW, dim_r))
    freqs = np.concatenate([fH, fW], axis=-1).reshape(N, ROT)      # (1024, 32)

    cos_d = np.full((128, N), 1.0 / SW, np.float32)
    sin_d = np.zeros((128, N), np.float32)
    ct = np.cos(freqs).T.astype(np.float32) / SW      # (32, 1024)
    st = np.sin(freqs).T.astype(np.float32) / SW
    sgn = np.where(np.arange(ROT) % 2 == 0, -1.0, 1.0)[:, None].astype(np.float32)
    cos_d[0:32] = ct
    cos_d[64:96] = ct
    sin_d[0:32] = st * sgn
    sin_d[64:96] = st * sgn
    return (cos_d.astype(ml_dtypes.bfloat16).view(np.uint16),
            sin_d.astype(ml_dtypes.bfloat16).view(np.uint16))


def _pair_layout(a):
    """[1024, X] c-major -> [128, 4, 2, X] (partition, k-block-pair, tile)."""
    X = a.shape[1]
    return np.ascontiguousarray(a.reshape(4, 2, 128, X).transpose(2, 0, 1, 3))


def _fp8_split_x(a):
    """hi, unscaled residual, and a/SL, all e4m3.

    The residual stays unscaled so all three QKV terms accumulate into one
    PSUM at the same scale (the w residual is SL-scaled against a/SL).
    """
    f8 = ml_dtypes.float8_e4m3fn
    hi = a.astype(f8)
    lo = (a - hi.astype(np.float32)).astype(f8)
    sm = (a / SL).astype(f8)
    return hi, lo, sm


def _fp8_split_w(a):
    """hi and SL-scaled residual, e4m3 (pairs with the a/SL x operand)."""
    f8 = ml_dtypes.float8_e4m3fn
    hi = a.astype(f8)
    lo = ((a - hi.astype(np.float32)) * SL).astype(f8)
    return hi, lo


def _build_program():
    nc = bacc_mod.Bacc()
    xh_h = nc.declare_dram_parameter("x_hi", [128, 8192], U8, isOutput=False)
    xl_h = nc.declare_dram_parameter("x_lo", [128, 8192], U8, isOutput=False)
    xs_h = nc.declare_dram_parameter("x_sm", [128, 8192], U8, isOutput=False)
    wh_h = nc.declare_dram_parameter("w_hi", [6, 128, 4096], U8, isOutput=False)
    wl_h = nc.declare_dram_parameter("w_lo", [6, 128, 4096], U8, isOutput=False)
    wp_h = nc.declare_dram_parameter("w_proj16", [C, C], U16, isOutput=False)
    brow_h = nc.declare_dram_parameter("b_row", [1, C], F32, isOutput=False)
    cos_h = nc.declare_dram_parameter("cos_d", [128, N], U16, isOutput=False)
    sin_h = nc.declare_dram_parameter("sin_d", [128, N], U16, isOutput=False)
    out_h = nc.declare_dram_parameter("out", [N, C], F32, isOutput=True)

    def f32r(ap):
        return ap.bitcast(F32R)

    DR = mybir.MatmulPerfMode.DoubleRow
    MUL = mybir.AluOpType.mult
    ADD = mybir.AluOpType.add

    with nc.allow_low_precision(reason="fp8/bf16 operands within rel-err gate"), \
         TileContext(nc) as tc, \
         tc.tile_pool(name="consts", bufs=1) as consts, \
         tc.tile_pool(name="big", bufs=1) as big, \
         tc.tile_pool(name="wq", bufs=3) as wq, \
         tc.tile_pool(name="rot", bufs=2) as rot, \
         tc.tile_pool(name="expp", bufs=6) as expp, \
         tc.tile_pool(name="navp", bufs=2) as navp, \
         tc.tile_pool(name="yout", bufs=2) as yout:

        cos_sb = consts.tile([128, N], BF16)
        sin_sb = consts.tile([128, N], BF16)
        brow_sb = consts.tile([1, C], F32)
        bias_bc = consts.tile([128, C], F32)

        # persistent activations
        xh_sb = big.tile([128, 4, 2, N], FP8)
        xl_sb = big.tile([128, 4, 2, N], FP8)
        xs_sb = big.tile([128, 4, 2, N], FP8)
        qrot_sb = big.tile([128, 8, N], BF16)      # Q_rot^T  (d-major)
        krot_sb = big.tile([128, 8, N], BF16)      # K_rot^T
        vext_sb = big.tile([128, 8, 16, 65], BF16)  # V | SW, per tok-block
        attn_sb = big.tile([128, 8, N], BF16)      # attn_out^T (c-major)
        wp_sb = big.tile([128, 8, C], BF16)        # w_proj rows

        # ---- DMA stream (sync/HWDGE, ordered = arrival order) ----
        def dma_x1(dst, src, kbp):
            nc.sync.dma_start(
                out=dst[:, kbp, :, :].rearrange("p a b -> p (a b)").bitcast(U8),
                in_=src[:, kbp * 2048:(kbp + 1) * 2048],
            )

        w_tiles = {}

        def dma_w(og, split=False):
            whi = wq.tile([128, 4, 2, 512], FP8, tag="whi", name=f"whi{og}")
            wlo = wq.tile([128, 4, 2, 512], FP8, tag="wlo", name=f"wlo{og}")
            w_tiles[og] = (whi, wlo)
            parts = ((0, 2), (2, 4)) if split else ((0, 4),)
            aps = []
            for t, h in ((whi, wh_h), (wlo, wl_h)):
                for a, b in parts:
                    aps.append((
                        t[:, a:b, :, :].rearrange("p a b c -> p (a b c)").bitcast(U8),
                        h[og, :, a * 1024:b * 1024],
                    ))
            return aps

        # first V block needs whi4/wlo4 kbp 0-1 and the kbp-0 x chunks; order
        # the queue so the PE can start ~2us in and stays fed kbp-major.
        w4 = dma_w(4, split=True)   # [hi01, hi23, lo01, lo23]
        nc.sync.dma_start(out=w4[0][0], in_=w4[0][1])
        dma_x1(xh_sb, xh_h, 0)
        nc.sync.dma_start(out=w4[2][0], in_=w4[2][1])
        dma_x1(xl_sb, xl_h, 0)
        dma_x1(xs_sb, xs_h, 0)
        nc.sync.dma_start(out=w4[1][0], in_=w4[1][1])
        nc.sync.dma_start(out=w4[3][0], in_=w4[3][1])
        for kbp in range(1, 4):
            dma_x1(xh_sb, xh_h, kbp)
            dma_x1(xl_sb, xl_h, kbp)
            dma_x1(xs_sb, xs_h, kbp)
        for og in (5, 0, 2):
            for o, i in dma_w(og):
                nc.sync.dma_start(out=o, in_=i)
        nc.sync.dma_start(out=cos_sb.bitcast(U16), in_=cos_h[:, :])
        nc.sync.dma_start(out=sin_sb.bitcast(U16), in_=sin_h[:, :])
        nc.sync.dma_start(out=f32r(brow_sb), in_=f32r(brow_h[:, :]))
        for og in (1, 3):
            for o, i in dma_w(og):
                nc.sync.dma_start(out=o, in_=i)
        for cb in range(8):
            nc.sync.dma_start(
                out=wp_sb[:, cb, :].bitcast(U16),
                in_=wp_h[cb * 128:(cb + 1) * 128, :],
            )

        # ones(SW) column of V_ext; bias broadcast row
        nc.gpsimd.memset(vext_sb[:, :, :, 64:65], SW)
        nc.gpsimd.partition_broadcast(bias_bc, brow_sb)

        with tc.tile_pool(name="ps_lg", bufs=2, space="PSUM") as ps_lg, \
             tc.tile_pool(name="ps_av", bufs=2, space="PSUM") as ps_av:

            # ---------- QKV (fp8 DoubleRow, 3 terms, one PSUM) ----------
            def qkv_block(ps_m, og, j, col0):
                """One [128, 512] out chunk.

                q/k ogs (0-3): out dims = w cols (j), cols = tokens col0..+512.
                v ogs (4,5): out dims = tokens (j = tb), cols = w cols col0..+512.
                """
                whi, wlo = w_tiles[og]
                qk = og < 4
                for cc in range(2):
                    dm = ps_m[:, cc * 256:(cc + 1) * 256]
                    if qk:
                        wsl = lambda w: w[:, kbp, :, j * 128:(j + 1) * 128]
                        xsl = lambda x: x[:, kbp, :, col0 + cc * 256:col0 + (cc + 1) * 256]
                        terms = [(whi, xh_sb), (whi, xl_sb), (wlo, xs_sb)]
                    else:
                        xsl = lambda x: x[:, kbp, :, j * 128:(j + 1) * 128]
                        wsl = lambda w: w[:, kbp, :, col0 + cc * 256:col0 + (cc + 1) * 256]
                        terms = [(xh_sb, whi), (xl_sb, whi), (xs_sb, wlo)]
                    for ti, (lt, rt) in enumerate(terms):
                        for kbp in range(4):
                            lhs = wsl(lt) if qk else xsl(lt)
                            rhs = xsl(rt) if qk else wsl(rt)
                            nc.tensor.matmul(
                                dm, lhs, rhs,
                                start=(ti == 0 and kbp == 0),
                                stop=(ti == 2 and kbp == 3),
                                perf_mode=DR,
                            )

            def rotary(q_sb, dst):
                """q_sb [128,1024] bf16 (SW-scaled) -> dst = rope(q)/SW."""
                shuf = rot.tile([128, N], BF16, tag="shuf")
                nc.vector.stream_shuffle(shuf, q_sb, PAIRMASK)
                tmp = rot.tile([128, N], BF16, tag="tmp")
                nc.vector.tensor_mul(tmp, shuf, sin_sb)
                nc.vector.tensor_mul(dst, q_sb, cos_sb)
                nc.vector.tensor_add(dst, dst, tmp)

            # ---------- attention ----------
            def attention(h, sig, fillers, nf=2):
                hp, r0 = h // 2, (h % 2) * 64
                q0 = sig * 512
                es = []
                for ktp in range(4):
                    lg = ps_lg.tile([128, 2, 512], F32, tag="lg",
                                    name=f"lg{h}_{sig}_{ktp}")
                    for i in range(2):
                        kt = ktp * 2 + i
                        nc.tensor.matmul(
                            lg[:, i, :],
                            krot_sb[r0:r0 + 64, hp, kt * 128:(kt + 1) * 128],
                            qrot_sb[r0:r0 + 64, hp, q0:q0 + 512],
                            start=True, stop=True,
                        )
                    e = expp.tile([128, 2, 512], BF16, tag="e",
                                  name=f"e{h}_{sig}_{ktp}")
                    nc.scalar.activation(
                        e.rearrange("p a b -> p (a b)"),
                        lg.rearrange("p a b -> p (a b)"),
                        mybir.ActivationFunctionType.Exp, scale=0.125,
                    )
                    es.append(e)
                    if fillers and (ktp == 1 or (ktp == 3 and nf > 1)):
                        fillers.pop(0)()
                av = ps_av.tile([65, 512], F32, tag="av", name=f"av{h}_{sig}")
                for ktp in range(4):
                    for i in range(2):
                        kt = ktp * 2 + i
                        nc.tensor.matmul(
                            av, vext_sb[:, kt, h, 0:65], es[ktp][:, i, :],
                            start=(kt == 0), stop=(kt == 7),
                        )
                recip = navp.tile([1, 512], F32, tag="recip", bufs=1)
                nc.vector.reciprocal(recip, av[64:65, :])
                rb = navp.tile([64, 512], F32, tag="rb", bufs=1)
                nc.gpsimd.partition_broadcast(rb, recip)
                nc.vector.tensor_mul(
                    attn_sb[r0:r0 + 64, hp, q0:q0 + 512], av[0:64, :], rb
                )

            # ---------- era A: qkv + attention half 0 ----------
            with tc.tile_pool(name="ps_qm", bufs=2, space="PSUM") as ps_qm:

                def v_unit(og, tb):
                    def emit():
                        m = ps_qm.tile([128, 512], F32, tag="qm", name=f"vm{og}_{tb}")
                        qkv_block(m, og, tb, 0)
                        vh = og - 4
                        nc.scalar.copy(
                            vext_sb[:, tb, vh * 8:(vh + 1) * 8, 0:64],
                            m.rearrange("p (a b) -> p a b", a=8),
                        )
                    return emit

                qsb_tiles = {}

                def qk_unit(og, j, half):
                    """half 0/1 of tokens for q/k out-block j; rotary on half 1."""
                    def emit():
                        m = ps_qm.tile([128, 512], F32, tag="qm", name=f"qm{og}_{j}_{half}")
                        qkv_block(m, og, j, half * 512)
                        if half == 0:
                            qsb_tiles[(og, j)] = rot.tile(
                                [128, N], BF16, tag="q_sb",
                                name=f"qsb{og}_{j}", bufs=2)
                        q_sb = qsb_tiles[(og, j)]
                        nc.scalar.copy(q_sb[:, half * 512:half * 512 + 512], m)
                        if half == 1:
                            dst = (qrot_sb if og in (0, 1) else krot_sb)
                            hp = j + (4 if og in (1, 3) else 0)
                            rotary(q_sb, dst[:, hp, :])
                    return emit

                # V first, then q/k heads 0-7
                for og in (4, 5):
                    for tb in range(8):
                        v_unit(og, tb)()
                for j in range(4):
                    for og in (0, 2):
                        qk_unit(og, j, 0)()
                        qk_unit(og, j, 1)()

                # half 0, with q/k heads 8-15 spread as PE fillers across
                # heads 0-13 (1/head; 2 for h0,1): attention alone is
                # ACT-paced, so every head needs some PE filler.
                fillers = []
                for j in range(4):
                    for og in (1, 3):
                        fillers.append(qk_unit(og, j, 0))
                        fillers.append(qk_unit(og, j, 1))
                for h in range(16):
                    attention(h, 0, fillers, nf=(2 if h < 2 else 1))
                while fillers:
                    fillers.pop(0)()

            # ---------- era B: attention half 1 + proj half 0 ----------
            def proj_chunk(ctx, cb):
                y, qb = ctx
                for oc in range(2):
                    nc.tensor.matmul(
                        y[:, oc, :],
                        attn_sb[:, cb, qb * 128:(qb + 1) * 128],
                        wp_sb[:, cb, oc * 512:(oc + 1) * 512],
                        start=(cb == 0), stop=(cb == 7),
                    )
                if cb == 7:
                    y_sb = yout.tile([128, C], F32, tag="y_sb", name=f"ysb{qb}")
                    nc.vector.scalar_tensor_tensor(
                        out=y_sb, in0=y.rearrange("p a b -> p (a b)"),
                        scalar=1.0, in1=bias_bc, op0=MUL, op1=ADD,
                    )
                    nc.sync.dma_start(
                        out=out_h[qb * 128:(qb + 1) * 128, :], in_=y_sb
                    )

            with tc.tile_pool(name="ps_y", bufs=1, space="PSUM") as ps_y:
                fillers = []
                for qb in range(4):
                    ctx = None
                    for cb in range(8):
                        def emit(qb=qb, cb=cb):
                            nonlocal ctx
                            if cb == 0:
                                ctx = (ps_y.tile([128, 2, 512], F32, tag="y",
                                                 name=f"y{qb}"), qb)
                            proj_chunk(ctx, cb)
                        fillers.append(emit)
                for h in range(16):
                    attention(h, 1, fillers)
                while fillers:
                    fillers.pop(0)()

        # ---------- tail: proj half 1 (lg/av freed; double-buffered) ----------
        with tc.tile_pool(name="ps_y2", bufs=2, space="PSUM") as ps_y2:
            for qb in range(4, 8):
                y = ps_y2.tile([128, 2, 512], F32, tag="y", name=f"y{qb}")
                for cb in range(8):
                    proj_chunk((y, qb), cb)

    nc.finalize()
    return nc


_PROGRAM = None


def kernel(x, w_qkv, w_proj, b_proj):
    global _PROGRAM
    if _PROGRAM is None:
        _PROGRAM = _build_program()
    nc = _PROGRAM

    cos_d, sin_d = _host_tables()
    wq_s = np.asarray(w_qkv, np.float32) * SW
    whs, wls = [], []
    for og in range(6):
        wro = _pair_layout(wq_s[:, og * 512:(og + 1) * 512])
        hi, lo = _fp8_split_w(wro)
        whs.append(hi.reshape(128, 4096).view(np.uint8))
        wls.append(lo.reshape(128, 4096).view(np.uint8))
    shared = {
        "w_hi": np.ascontiguousarray(np.stack(whs)),
        "w_lo": np.ascontiguousarray(np.stack(wls)),
        "w_proj16": np.ascontiguousarray(
            np.asarray(w_proj, np.float32).astype(ml_dtypes.bfloat16).view(np.uint16)),
        "b_row": np.ascontiguousarray(b_proj, np.float32).reshape(1, C),
        "cos_d": cos_d,
        "sin_d": sin_d,
    }
    in_maps = []
    for b in range(NCORES):
        xr = _pair_layout(np.ascontiguousarray(np.asarray(x[b], np.float32).T))
        hi, lo, sm = _fp8_split_x(xr)
        in_maps.append({
            "x_hi": np.ascontiguousarray(hi.reshape(128, 8192).view(np.uint8)),
            "x_lo": np.ascontiguousarray(lo.reshape(128, 8192).view(np.uint8)),
            "x_sm": np.ascontiguousarray(sm.reshape(128, 8192).view(np.uint8)),
            **shared,
        })
    res = run_bass_kernel_spmd(nc, in_maps, core_ids=list(range(NCORES)))
    return np.stack([res.results[b]["out"] for b in range(NCORES)], axis=0)


if __name__ == "__main__":
    xs = np.random.randn(B, N, C).astype(np.float32)
    wq = (np.random.randn(C, 3 * C) / np.sqrt(C)).astype(np.float32)
    wp = (np.random.randn(C, C) / np.sqrt(C)).astype(np.float32)
    bp = (np.random.randn(C) * 0.01).astype(np.float32)
    out = kernel(x=xs, w_qkv=wq, w_proj=wp, b_proj=bp)
    print(out.shape, out.dtype)


# revision 21
# speedup vs baseline: 1.3960x; 1.0258x over previous
"""Trainium2 Bass kernel for nn_Attention_5514738008849.

Dense transformer attention block with axial rotary embeddings:
  x:(8,1024,1024) -> qkv -> rope(q,k) -> softmax(qk^T/sqrt(d)) v -> proj+bias

Sharding: pure data-parallel over batch B=8 across the 8 NeuronCores (one
batch element per core, full weights replicated). No collectives.

Per-core dataflow:
  - QKV runs as fp8e4 DoubleRow matmuls (0.5 cyc/row, K=256 per pass) using a
    3-term hi/lo residual split of both x and w_qkv (host-precomputed):
        x@w ~= x_hi@w_hi + (x_lo@w_hi)/64 + (x_hi/64)@(w_lo*64)
    with w globally prescaled by 16 for fp8 range; terms 1+3 accumulate in one
    PSUM, term 2 in a second, merged at evacuation on GpSimd with the 1/64.
  - rotary: DVE stream_shuffle pair-swaps partitions; the sign and the 1/16
    w-descale fold into the host cos/sin tables; bf16 combine runs at DVE 2x.
  - logits^T[k,q] per head in bf16; exp on ACT (scale=1/8) -- ACT runs only
    the exps (its throughput is the attention-phase floor), all PSUM
    evacuations ride on GpSimd.
  - AV in bf16 with a 16.0-column appended to V so row 64 of the accumulator
    carries the (16x-scaled) softmax denominator; normalize = DVE reciprocal +
    GpSimd partition_broadcast + DVE multiply straight out of PSUM.
  - attention runs in q-halves: during half 1, proj of half 0 fills the PE
    while ACT exps; QKV for heads 8-15 interleaves into heads 0-7's half 0.
  - proj token-major in bf16; bias fused into the GpSimd PSUM evacuation.
"""

import os
import sys

sys.path.insert(0, "/opt/trn_rl_repo")

# This kernel needs the axon-tunneled NeuronCores. A JAX_PLATFORMS=cpu pin
# (used by some harnesses for the jax reference) would prevent the axon
# backend from registering; clearing it here is a no-op when jax has already
# initialized and restores device visibility when it hasn't.
if os.environ.get("JAX_PLATFORMS", "") not in ("", None):
    if "axon" not in os.environ["JAX_PLATFORMS"]:
        os.environ.pop("JAX_PLATFORMS", None)

import numpy as np
import ml_dtypes

import concourse.bass as bass
import concourse.bacc as bacc_mod
import concourse.mybir as mybir
from concourse.bass_utils import run_bass_kernel_spmd
from concourse.tile import TileContext

B, N, C = 8, 1024, 1024
H, D = 16, 64          # heads, head dim
ROT = 32               # rotary dims per head (head_dim // 2)
FH = FW = 32           # token grid for axial rope
NCORES = 8
F32 = mybir.dt.float32
F32R = mybir.dt.float32r
BF16 = mybir.dt.bfloat16
FP8 = mybir.dt.float8e4
U8 = mybir.dt.uint8
U16 = mybir.dt.uint16

SW = 16.0              # global w_qkv prescale for fp8 range
SL = 64.0              # hi/lo residual scale

PAIRMASK = [i ^ 1 for i in range(32)]   # stream_shuffle partition pair swap


def _host_tables():
    """Rotary cos/sin tables, d-major (dim-on-partition), bf16.

    The stream_shuffle is a plain pair swap, so the rotate-half sign lives in
    the sin table (-sin on even rows, +sin on odd rows), and the 1/SW descale
    of the fp8-scaled QKV results is folded into both tables.
    """
    dim_r = D // 4                                    # 16
    base = np.linspace(1.0, (FH * FW) / 2.0, dim_r // 2) * np.pi   # (8,)

    def axis_freqs(n):
        pos = np.linspace(-1.0, 1.0, n)
        f = pos[:, None] * base[None, :]              # (n, 8)
        return np.repeat(f, 2, axis=-1)               # (n, 16)

    fH = np.broadcast_to(axis_freqs(FH)[:, None, :], (FH, FW, dim_r))
    fW = np.broadcast_to(axis_freqs(FW)[None, :, :], (FH, FW, dim_r))
    freqs = np.concatenate([fH, fW], axis=-1).reshape(N, ROT)      # (1024, 32)

    cos_d = np.full((128, N), 1.0 / SW, np.float32)
    sin_d = np.zeros((128, N), np.float32)
    ct = np.cos(freqs).T.astype(np.float32) / SW      # (32, 1024)
    st = np.sin(freqs).T.astype(np.float32) / SW
    sgn = np.where(np.arange(ROT) % 2 == 0, -1.0, 1.0)[:, None].astype(np.float32)
    cos_d[0:32] = ct
    cos_d[64:96] = ct
    sin_d[0:32] = st * sgn
    sin_d[64:96] = st * sgn
    return (cos_d.astype(ml_dtypes.bfloat16).view(np.uint16),
            sin_d.astype(ml_dtypes.bfloat16).view(np.uint16))


def _pair_layout(a):
    """[1024, X] c-major -> [128, 4, 2, X] (partition, k-block-pair, tile)."""
    X = a.shape[1]
    return np.ascontiguousarray(a.reshape(4, 2, 128, X).transpose(2, 0, 1, 3))


def _fp8_split_x(a):
    """hi, unscaled residual, and a/SL, all e4m3.

    The residual stays unscaled so all three QKV terms accumulate into one
    PSUM at the same scale (the w residual is SL-scaled against a/SL).
    """
    f8 = ml_dtypes.float8_e4m3fn
    hi = a.astype(f8)
    lo = (a - hi.astype(np.float32)).astype(f8)
    sm = (a / SL).astype(f8)
    return hi, lo, sm


def _fp8_split_w(a):
    """hi and SL-scaled residual, e4m3 (pairs with the a/SL x operand)."""
    f8 = ml_dtypes.float8_e4m3fn
    hi = a.astype(f8)
    lo = ((a - hi.astype(np.float32)) * SL).astype(f8)
    return hi, lo


def _build_program():
    nc = bacc_mod.Bacc()
    xh_h = nc.declare_dram_parameter("x_hi", [128, 8192], U8, isOutput=False)
    xl_h = nc.declare_dram_parameter("x_lo", [128, 8192], U8, isOutput=False)
    xs_h = nc.declare_dram_parameter("x_sm", [128, 8192], U8, isOutput=False)
    wh_h = nc.declare_dram_parameter("w_hi", [6, 128, 4096], U8, isOutput=False)
    wl_h = nc.declare_dram_parameter("w_lo", [6, 128, 4096], U8, isOutput=False)
    wp_h = nc.declare_dram_parameter("w_proj16", [C, C], U16, isOutput=False)
    brow_h = nc.declare_dram_parameter("b_row", [1, C], F32, isOutput=False)
    cos_h = nc.declare_dram_parameter("cos_d", [128, N], U16, isOutput=False)
    sin_h = nc.declare_dram_parameter("sin_d", [128, N], U16, isOutput=False)
    out_h = nc.declare_dram_parameter("out", [N, C], F32, isOutput=True)

    def f32r(ap):
        return ap.bitcast(F32R)

    DR = mybir.MatmulPerfMode.DoubleRow
    MUL = mybir.AluOpType.mult
    ADD = mybir.AluOpType.add

    with nc.allow_low_precision(reason="fp8/bf16 operands within rel-err gate"), \
         TileContext(nc) as tc, \
         tc.tile_pool(name="consts", bufs=1) as consts, \
         tc.tile_pool(name="big", bufs=1) as big, \
         tc.tile_pool(name="wq", bufs=3) as wq, \
         tc.tile_pool(name="rot", bufs=2) as rot, \
         tc.tile_pool(name="expp", bufs=6) as expp, \
         tc.tile_pool(name="navp", bufs=2) as navp, \
         tc.tile_pool(name="yout", bufs=2) as yout:

        cos_sb = consts.tile([128, N], BF16)
        sin_sb = consts.tile([128, N], BF16)
        brow_sb = consts.tile([1, C], F32)
        bias_bc = consts.tile([128, C], F32)

        # persistent activations
        xh_sb = big.tile([128, 4, 2, N], FP8)
        xl_sb = big.tile([128, 4, 2, N], FP8)
        xs_sb = big.tile([128, 4, 2, N], FP8)
        qrot_sb = big.tile([128, 8, N], BF16)      # Q_rot^T  (d-major)
        krot_sb = big.tile([128, 8, N], BF16)      # K_rot^T
        vext_sb = big.tile([128, 8, 16, 65], BF16)  # V | SW, per tok-block
        attn_sb = big.tile([128, 8, N], BF16)      # attn_out^T (c-major)
        wp_sb = big.tile([128, 8, C], BF16)        # w_proj rows

        # ---- DMA stream (sync/HWDGE, ordered = arrival order) ----
        def dma_x1(dst, src, kbp):
            nc.sync.dma_start(
                out=dst[:, kbp, :, :].rearrange("p a b -> p (a b)").bitcast(U8),
                in_=src[:, kbp * 2048:(kbp + 1) * 2048],
            )

        w_tiles = {}

        def dma_w(og, split=False):
            whi = wq.tile([128, 4, 2, 512], FP8, tag="whi", name=f"whi{og}")
            wlo = wq.tile([128, 4, 2, 512], FP8, tag="wlo", name=f"wlo{og}")
            w_tiles[og] = (whi, wlo)
            parts = ((0, 2), (2, 4)) if split else ((0, 4),)
            aps = []
            for t, h in ((whi, wh_h), (wlo, wl_h)):
                for a, b in parts:
                    aps.append((
                        t[:, a:b, :, :].rearrange("p a b c -> p (a b c)").bitcast(U8),
                        h[og, :, a * 1024:b * 1024],
                    ))
            return aps

        # The V sweep consumes x kbp-major; order the queue so each kbp's
        # (w_hi, x_hi, x_lo, w_lo, x_sm) lands just ahead of its matmuls.
        w4 = dma_w(4, split=True)   # [hi01, hi23, lo01, lo23]
        nc.sync.dma_start(out=w4[0][0], in_=w4[0][1])
        dma_x1(xh_sb, xh_h, 0)
        dma_x1(xl_sb, xl_h, 0)
        nc.sync.dma_start(out=w4[2][0], in_=w4[2][1])
        dma_x1(xs_sb, xs_h, 0)
        for kbp in (1, 2):
            dma_x1(xh_sb, xh_h, kbp)
            dma_x1(xl_sb, xl_h, kbp)
            dma_x1(xs_sb, xs_h, kbp)
            if kbp == 1:
                nc.sync.dma_start(out=w4[1][0], in_=w4[1][1])
                nc.sync.dma_start(out=w4[3][0], in_=w4[3][1])
        dma_x1(xh_sb, xh_h, 3)
        dma_x1(xl_sb, xl_h, 3)
        dma_x1(xs_sb, xs_h, 3)
        for og in (5, 0, 2):
            for o, i in dma_w(og):
                nc.sync.dma_start(out=o, in_=i)
        nc.sync.dma_start(out=cos_sb.bitcast(U16), in_=cos_h[:, :])
        nc.sync.dma_start(out=sin_sb.bitcast(U16), in_=sin_h[:, :])
        nc.sync.dma_start(out=f32r(brow_sb), in_=f32r(brow_h[:, :]))
        for og in (1, 3):
            for o, i in dma_w(og):
                nc.sync.dma_start(out=o, in_=i)
        for cb in range(8):
            nc.sync.dma_start(
                out=wp_sb[:, cb, :].bitcast(U16),
                in_=wp_h[cb * 128:(cb + 1) * 128, :],
            )

        # ones(SW) column of V_ext; bias broadcast row
        nc.gpsimd.memset(vext_sb[:, :, :, 64:65], SW)
        nc.gpsimd.partition_broadcast(bias_bc, brow_sb)

        # ---------- V (og 4,5): kbp-major sweep so the PE starts on the
        # first-arriving x chunks and never head-of-line blocks on later
        # kbp operands still in flight ----------
        with tc.tile_pool(name="ps_v", bufs=6, space="PSUM") as ps_v:
            for og in (4, 5):
                whi, wlo = w_tiles[og]
                for tbg in (0, 4):
                    tiles = {}
                    for kbp in range(4):
                        for tb in range(tbg, tbg + 4):
                            if kbp == 0:
                                tiles[tb] = ps_v.tile(
                                    [128, 512], F32, tag="vps",
                                    name=f"v{og}_{tb}")
                            m = tiles[tb]
                            # one start per PSUM bank: a start marks the whole
                            # 2KB zero region, so the sibling cc chunk must
                            # not re-start after this chunk has accumulated
                            for ti, (lt, rt) in enumerate(
                                    ((xh_sb, whi), (xl_sb, whi), (xs_sb, wlo))):
                                for cc in range(2):
                                    nc.tensor.matmul(
                                        m[:, cc * 256:(cc + 1) * 256],
                                        lt[:, kbp, :, tb * 128:(tb + 1) * 128],
                                        rt[:, kbp, :, cc * 256:(cc + 1) * 256],
                                        start=(kbp == 0 and ti == 0 and cc == 0),
                                        stop=(kbp == 3 and ti == 2),
                                        perf_mode=DR,
                                        skip_group_check=True,
                                    )
                            if kbp == 3:
                                vh = og - 4
                                nc.scalar.copy(
                                    vext_sb[:, tb, vh * 8:(vh + 1) * 8, 0:64],
                                    m.rearrange("p (a b) -> p a b", a=8),
                                )

        with tc.tile_pool(name="ps_lg", bufs=2, space="PSUM") as ps_lg, \
             tc.tile_pool(name="ps_av", bufs=2, space="PSUM") as ps_av:

            # ---------- QKV (fp8 DoubleRow, 3 terms, one PSUM) ----------
            def qkv_block(ps_m, og, j, col0):
                """One [128, 512] out chunk.

                q/k ogs (0-3): out dims = w cols (j), cols = tokens col0..+512.
                v ogs (4,5): out dims = tokens (j = tb), cols = w cols col0..+512.
                """
                whi, wlo = w_tiles[og]
                qk = og < 4
                for cc in range(2):
                    dm = ps_m[:, cc * 256:(cc + 1) * 256]
                    if qk:
                        wsl = lambda w: w[:, kbp, :, j * 128:(j + 1) * 128]
                        xsl = lambda x: x[:, kbp, :, col0 + cc * 256:col0 + (cc + 1) * 256]
                        terms = [(whi, xh_sb), (whi, xl_sb), (wlo, xs_sb)]
                    else:
                        xsl = lambda x: x[:, kbp, :, j * 128:(j + 1) * 128]
                        wsl = lambda w: w[:, kbp, :, col0 + cc * 256:col0 + (cc + 1) * 256]
                        terms = [(xh_sb, whi), (xl_sb, whi), (xs_sb, wlo)]
                    for ti, (lt, rt) in enumerate(terms):
                        for kbp in range(4):
                            lhs = wsl(lt) if qk else xsl(lt)
                            rhs = xsl(rt) if qk else wsl(rt)
                            nc.tensor.matmul(
                                dm, lhs, rhs,
                                start=(ti == 0 and kbp == 0),
                                stop=(ti == 2 and kbp == 3),
                                perf_mode=DR,
                            )

            def rotary(q_sb, dst):
                """q_sb [128,1024] bf16 (SW-scaled) -> dst = rope(q)/SW."""
                shuf = rot.tile([128, N], BF16, tag="shuf")
                nc.vector.stream_shuffle(shuf, q_sb, PAIRMASK)
                tmp = rot.tile([128, N], BF16, tag="tmp")
                nc.vector.tensor_mul(tmp, shuf, sin_sb)
                nc.vector.tensor_mul(dst, q_sb, cos_sb)
                nc.vector.tensor_add(dst, dst, tmp)

            # ---------- attention ----------
            def attention(h, sig, fillers, nf=2):
                hp, r0 = h // 2, (h % 2) * 64
                q0 = sig * 512
                es = []
                for ktp in range(4):
                    lg = ps_lg.tile([128, 2, 512], F32, tag="lg",
                                    name=f"lg{h}_{sig}_{ktp}")
                    for i in range(2):
                        kt = ktp * 2 + i
                        nc.tensor.matmul(
                            lg[:, i, :],
                            krot_sb[r0:r0 + 64, hp, kt * 128:(kt + 1) * 128],
                            qrot_sb[r0:r0 + 64, hp, q0:q0 + 512],
                            start=True, stop=True,
                        )
                    e = expp.tile([128, 2, 512], BF16, tag="e",
                                  name=f"e{h}_{sig}_{ktp}")
                    nc.scalar.activation(
                        e.rearrange("p a b -> p (a b)"),
                        lg.rearrange("p a b -> p (a b)"),
                        mybir.ActivationFunctionType.Exp, scale=0.125,
                    )
                    es.append(e)
                    if fillers and (ktp == 1 or (ktp == 3 and nf > 1)):
                        fillers.pop(0)()
                av = ps_av.tile([65, 512], F32, tag="av", name=f"av{h}_{sig}")
                for ktp in range(4):
                    for i in range(2):
                        kt = ktp * 2 + i
                        nc.tensor.matmul(
                            av, vext_sb[:, kt, h, 0:65], es[ktp][:, i, :],
                            start=(kt == 0), stop=(kt == 7),
                        )
                recip = navp.tile([1, 512], F32, tag="recip", bufs=1)
                nc.vector.reciprocal(recip, av[64:65, :])
                rb = navp.tile([64, 512], F32, tag="rb", bufs=1)
                nc.gpsimd.partition_broadcast(rb, recip)
                nc.vector.tensor_mul(
                    attn_sb[r0:r0 + 64, hp, q0:q0 + 512], av[0:64, :], rb
                )

            # ---------- era A: qkv + attention half 0 ----------
            with tc.tile_pool(name="ps_qm", bufs=2, space="PSUM") as ps_qm:

                qsb_tiles = {}

                def qk_unit(og, j, half):
                    """half 0/1 of tokens for q/k out-block j; rotary on half 1."""
                    def emit():
                        m = ps_qm.tile([128, 512], F32, tag="qm", name=f"qm{og}_{j}_{half}")
                        qkv_block(m, og, j, half * 512)
                        if half == 0:
                            qsb_tiles[(og, j)] = rot.tile(
                                [128, N], BF16, tag="q_sb",
                                name=f"qsb{og}_{j}", bufs=2)
                        q_sb = qsb_tiles[(og, j)]
                        nc.scalar.copy(q_sb[:, half * 512:half * 512 + 512], m)
                        if half == 1:
                            dst = (qrot_sb if og in (0, 1) else krot_sb)
                            hp = j + (4 if og in (1, 3) else 0)
                            rotary(q_sb, dst[:, hp, :])
                    return emit

                # q/k heads 0-7
                for j in range(4):
                    for og in (0, 2):
                        qk_unit(og, j, 0)()
                        qk_unit(og, j, 1)()

                # half 0, with q/k heads 8-15 spread as PE fillers across
                # heads 0-13 (1/head; 2 for h0,1): attention alone is
                # ACT-paced, so every head needs some PE filler.
                fillers = []
                for j in range(4):
                    for og in (1, 3):
                        fillers.append(qk_unit(og, j, 0))
                        fillers.append(qk_unit(og, j, 1))
                for h in range(16):
                    attention(h, 0, fillers, nf=(2 if h < 2 else 1))
                while fillers:
                    fillers.pop(0)()

            # ---------- era B: attention half 1 + proj half 0 ----------
            def proj_chunk(ctx, cb):
                y, qb = ctx
                for oc in range(2):
                    nc.tensor.matmul(
                        y[:, oc, :],
                        attn_sb[:, cb, qb * 128:(qb + 1) * 128],
                        wp_sb[:, cb, oc * 512:(oc + 1) * 512],
                        start=(cb == 0), stop=(cb == 7),
                    )
                if cb == 7:
                    y_sb = yout.tile([128, C], F32, tag="y_sb", name=f"ysb{qb}")
                    nc.vector.scalar_tensor_tensor(
                        out=y_sb, in0=y.rearrange("p a b -> p (a b)"),
                        scalar=1.0, in1=bias_bc, op0=MUL, op1=ADD,
                    )
                    nc.sync.dma_start(
                        out=out_h[qb * 128:(qb + 1) * 128, :], in_=y_sb
                    )

            with tc.tile_pool(name="ps_y", bufs=1, space="PSUM") as ps_y:
                fillers = []
                for qb in range(4):
                    ctx = None
                    for cb in range(8):
                        def emit(qb=qb, cb=cb):
                            nonlocal ctx
                            if cb == 0:
                                ctx = (ps_y.tile([128, 2, 512], F32, tag="y",
                                                 name=f"y{qb}"), qb)
                            proj_chunk(ctx, cb)
                        fillers.append(emit)
                for h in range(16):
                    attention(h, 1, fillers)
                while fillers:
                    fillers.pop(0)()

        # ---------- tail: proj half 1 (lg/av freed; double-buffered) ----------
        with tc.tile_pool(name="ps_y2", bufs=2, space="PSUM") as ps_y2:
            for qb in range(4, 8):
                y = ps_y2.tile([128, 2, 512], F32, tag="y", name=f"y{qb}")
                for cb in range(8):
                    proj_chunk((y, qb), cb)

    nc.finalize()
    return nc


_PROGRAM = None


def kernel(x, w_qkv, w_proj, b_proj):
    global _PROGRAM
    if _PROGRAM is None:
        _PROGRAM = _build_program()
    nc = _PROGRAM

    cos_d, sin_d = _host_tables()
    wq_s = np.asarray(w_qkv, np.float32) * SW
    whs, wls = [], []
    for og in range(6):
        wro = _pair_layout(wq_s[:, og * 512:(og + 1) * 512])
        hi, lo = _fp8_split_w(wro)
        whs.append(hi.reshape(128, 4096).view(np.uint8))
        wls.append(lo.reshape(128, 4096).view(np.uint8))
    shared = {
        "w_hi": np.ascontiguousarray(np.stack(whs)),
        "w_lo": np.ascontiguousarray(np.stack(wls)),
        "w_proj16": np.ascontiguousarray(
            np.asarray(w_proj, np.float32).astype(ml_dtypes.bfloat16).view(np.uint16)),
        "b_row": np.ascontiguousarray(b_proj, np.float32).reshape(1, C),
        "cos_d": cos_d,
        "sin_d": sin_d,
    }
    in_maps = []
    for b in range(NCORES):
        xr = _pair_layout(np.ascontiguousarray(np.asarray(x[b], np.float32).T))
        hi, lo, sm = _fp8_split_x(xr)
        in_maps.append({
            "x_hi": np.ascontiguousarray(hi.reshape(128, 8192).view(np.uint8)),
            "x_lo": np.ascontiguousarray(lo.reshape(128, 8192).view(np.uint8)),
            "x_sm": np.ascontiguousarray(sm.reshape(128, 8192).view(np.uint8)),
            **shared,
        })
    res = run_bass_kernel_spmd(nc, in_maps, core_ids=list(range(NCORES)))
    return np.stack([res.results[b]["out"] for b in range(NCORES)], axis=0)


if __name__ == "__main__":
    xs = np.random.randn(B, N, C).astype(np.float32)
    wq = (np.random.randn(C, 3 * C) / np.sqrt(C)).astype(np.float32)
    wp = (np.random.randn(C, C) / np.sqrt(C)).astype(np.float32)
    bp = (np.random.randn(C) * 0.01).astype(np.float32)
    out = kernel(x=xs, w_qkv=wq, w_proj=wp, b_proj=bp)
    print(out.shape, out.dtype)
